# revision 7
# baseline (speedup 1.0000x reference)
"""Grouped-experts SwiGLU MLP (DeepseekV3 style) for Trainium2, 8 NeuronCores.

Sharding: expert-parallel. Core e owns expert e's weights and its static
4096-token split. No collectives needed — token routing is the host-side
slice, outputs concatenate back in token order.

Per-core kernel (all matmuls in bf16 with fp32 PSUM accumulation):
  gT[h, t] = wg[d, h].T @ xT[d, t]      (accumulate over 16 d-chunks of 128)
  uT[h, t] = wu[d, h].T @ xT[d, t]
  hT[h, t] = silu(gT) * uT              (ACT silu + DVE mul, stored bf16)
  out[t, d] = hT[h, t].T @ wd[h, d]     (accumulate over 11 h-chunks of 128)

x is fed pre-transposed ([dim, tokens]) per core so the contraction dim sits
on SBUF partitions for both operands; weights are DMA-cast fp32->bf16 on
load and stay resident in SBUF for the whole kernel.
"""

import numpy as np

NUM_EXPERTS = 8
DIM = 2048
HIDDEN = 1408
T_E = 4096  # tokens per expert (static equal splits)

P = 128
TN = 512              # token group width (matmul moving dim)
NG = T_E // TN        # 8 token groups
DC = DIM // P         # 16 contraction chunks for the up/gate matmuls
HC = HIDDEN // P      # 11 contraction chunks for the down matmul
NDO = DIM // TN       # 4 output-dim blocks of 512

_nc_cache = []


def _build_program():
    import concourse.mybir as mybir
    import concourse.tile as tile
    from concourse import bacc

    fp32 = mybir.dt.float32
    bf16 = mybir.dt.bfloat16
    AF = mybir.ActivationFunctionType

    nc = bacc.Bacc("TRN2", target_bir_lowering=False, debug=False)

    xT = nc.dram_tensor("xt", [DIM, T_E], fp32, kind="ExternalInput")
    wg = nc.dram_tensor("wg", [DIM, HIDDEN], fp32, kind="ExternalInput")
    wu = nc.dram_tensor("wu", [DIM, HIDDEN], fp32, kind="ExternalInput")
    wd = nc.dram_tensor("wd", [HIDDEN, DIM], fp32, kind="ExternalInput")
    out = nc.dram_tensor("out", [T_E, DIM], fp32, kind="ExternalOutput")

    with tile.TileContext(nc) as tc:
        with (
            tc.tile_pool(name="wpool", bufs=1) as wpool,
            tc.tile_pool(name="xpool", bufs=1) as xpool,
            tc.tile_pool(name="hpool", bufs=1) as hpool,
            tc.tile_pool(name="spool", bufs=2) as spool,
            tc.tile_pool(name="opool", bufs=2) as opool,
            tc.tile_pool(name="psum", bufs=2, space="PSUM") as psum_pool,
        ):
            # Resident bf16 weights: [128, chunk, free] with the contraction
            # chunk index as the middle dim. DMA-cast fp32->bf16 (SWDGE).
            wg_sb = wpool.tile([P, DC, HIDDEN], bf16, tag="wg")
            wu_sb = wpool.tile([P, DC, HIDDEN], bf16, tag="wu")
            wd_sb = wpool.tile([P, HC, DIM], bf16, tag="wd")
            # Emission order matters for the SWDGE queue: the first matmul
            # needs wg + xt(group 0); wu is read ~3us later, wd not until
            # the first down-projection (~75us in). Load in that order.
            xt0_sb = xpool.tile([P, DC, TN], bf16, tag="xt")
            for c in range(DC):
                nc.gpsimd.dma_start(out=wg_sb[:, c, :], in_=wg[c * P:(c + 1) * P, :])
            for c in range(DC):
                nc.gpsimd.dma_start(out=xt0_sb[:, c, :], in_=xT[c * P:(c + 1) * P, 0:TN])
            for c in range(DC):
                nc.gpsimd.dma_start(out=wu_sb[:, c, :], in_=wu[c * P:(c + 1) * P, :])
            for c in range(HC):
                nc.gpsimd.dma_start(out=wd_sb[:, c, :], in_=wd[c * P:(c + 1) * P, :])

            for g in range(NG):
                # xT group [128, 16, 512] bf16, DMA-cast per d-chunk.
                if g == 0:
                    xt_sb = xt0_sb
                else:
                    xt_sb = xpool.tile([P, DC, TN], bf16, tag="xt")
                    for c in range(DC):
                        nc.gpsimd.dma_start(
                            out=xt_sb[:, c, :],
                            in_=xT[c * P:(c + 1) * P, g * TN:(g + 1) * TN],
                        )

                ht_sb = hpool.tile([P, HC, TN], bf16, tag="ht")
                for hh in range(HC):
                    pg = psum_pool.tile([P, TN], fp32, tag="pg")
                    pu = psum_pool.tile([P, TN], fp32, tag="pu")
                    for c in range(DC):
                        nc.tensor.matmul(
                            pg,
                            wg_sb[:, c, hh * P:(hh + 1) * P],
                            xt_sb[:, c, :],
                            start=(c == 0),
                            stop=(c == DC - 1),
                        )
                    for c in range(DC):
                        nc.tensor.matmul(
                            pu,
                            wu_sb[:, c, hh * P:(hh + 1) * P],
                            xt_sb[:, c, :],
                            start=(c == 0),
                            stop=(c == DC - 1),
                        )
                    # silu(g)*u = (g * sigmoid(g)) * u. Each DVE op reads at
                    # most one PSUM operand (HW limit NCC_IBVF027); Silu LUT
                    # isn't in CoreSim so sigmoid+mul keeps this sim-testable.
                    sig = spool.tile([P, TN], fp32, tag="sig")
                    sil = spool.tile([P, TN], fp32, tag="sil")
                    nc.scalar.activation(sig, pg, AF.Sigmoid)
                    nc.vector.tensor_mul(sil, pg, sig)
                    nc.vector.tensor_mul(ht_sb[:, hh, :], sil, pu)

                for tb in range(TN // P):
                    ot = opool.tile([P, DIM], fp32, tag="ot")
                    # hh-outer so one stationary hT load feeds 4 accumulating
                    # matmuls (one per dout block) -> 4x fewer LDWEIGHTS.
                    # po spans 4 PSUM banks; pg/pu take the other 4.
                    po = psum_pool.tile([P, NDO, TN], fp32, tag="po", bufs=1)
                    for hh in range(HC):
                        for do in range(NDO):
                            nc.tensor.matmul(
                                po[:, do, :],
                                ht_sb[:, hh, tb * P:(tb + 1) * P],
                                wd_sb[:, hh, do * TN:(do + 1) * TN],
                                start=(hh == 0),
                                stop=(hh == HC - 1),
                            )
                    for do in range(NDO):
                        nc.vector.tensor_copy(ot[:, do * TN:(do + 1) * TN], po[:, do, :])
                    t0 = g * TN + tb * P
                    nc.sync.dma_start(out=out[t0:t0 + P, :], in_=ot)

    nc.compile()
    return nc


def _get_program():
    if not _nc_cache:
        _nc_cache.append(_build_program())
    return _nc_cache[0]


def kernel(x, num_tokens_per_expert, w_gate, w_up, w_down, **_ignored):
    from concourse.bass_utils import run_bass_kernel_spmd

    x = np.asarray(x, dtype=np.float32)
    w_gate = np.asarray(w_gate, dtype=np.float32)
    w_up = np.asarray(w_up, dtype=np.float32)
    w_down = np.asarray(w_down, dtype=np.float32)

    nc = _get_program()

    xe = x.reshape(NUM_EXPERTS, T_E, DIM)
    in_maps = []
    for e in range(NUM_EXPERTS):
        in_maps.append(
            {
                "xt": np.ascontiguousarray(xe[e].T),
                "wg": np.ascontiguousarray(w_gate[e]),
                "wu": np.ascontiguousarray(w_up[e]),
                "wd": np.ascontiguousarray(w_down[e]),
            }
        )

    res = run_bass_kernel_spmd(nc, in_maps, core_ids=list(range(NUM_EXPERTS)))
    outs = [np.asarray(r["out"], dtype=np.float32) for r in res.results]
    return np.concatenate(outs, axis=0)
